# revision 4
# baseline (speedup 1.0000x reference)
"""Trainium2 Bass kernel for the Converter photometry problem.

Computes out = -2.5*log10(l_target @ (trans_filter * w).T) where w are
trapezoid quadrature weights derived from lam.  Data-parallel over 8
NeuronCores: l_target is sharded along batch B; the (small) weighted
filter matrix is replicated.

Per-core device program:
  - WT = (trans_filter * w).T  [L, F] resident in SBUF (host-prepared).
  - A shard [1024, 8192] streamed in 4MB DMA slabs, natural [b, l] layout.
  - PE transposes each [128b x 128l] tile (exact fp32 pass-through) into
    PSUM; ACT/DVE copy it back to SBUF as [l, b].
  - fp32r matmuls (1 cycle/row at N=512) with the WT k-chunk stationary
    and transposed-A moving accumulate flux.T [128f, 512b] in PSUM.
  - Ln + scale on eviction; per-core output is out.T [F, 1024].
Host reassembles the full [B, F] output.
"""

import math

import numpy as np

B, L, F = 8192, 8192, 128
N_CORES = 8
NB = B // N_CORES  # batch rows per core
P = 128
SBLK = 512  # output super-block along b (PSUM free dim)
W = 2048  # l window per DMA slab
UNIT_F_NU = 1.0673e-02
LOG10_SCALE = -2.5 / math.log(10.0)

_CACHE = {}


def _build_nc():
    import concourse.bacc as bacc
    import concourse.mybir as mybir
    from concourse import tile
    from concourse.masks import make_identity

    f32 = mybir.dt.float32
    f32r = mybir.dt.float32r

    KC = L // P  # 64 contraction chunks
    NSB = NB // SBLK  # 2 output super-blocks
    NW = L // W  # 4 l-windows
    CW = W // P  # 16 chunks per window
    T = SBLK // P  # 4 row sub-blocks per super-block

    nc = bacc.Bacc(None, target_bir_lowering=False, debug=False)
    a = nc.dram_tensor("a", [NB, L], f32, kind="ExternalInput")
    wt = nc.dram_tensor("wt", [L, F], f32, kind="ExternalInput")
    o = nc.dram_tensor("o", [F, NB], f32, kind="ExternalOutput")

    with tile.TileContext(nc) as tc:
        with (
            tc.tile_pool(name="const", bufs=1) as const_pool,
            tc.tile_pool(name="a_slab", bufs=2) as a_pool,
            tc.tile_pool(name="at", bufs=3) as at_pool,
            tc.tile_pool(name="psum_t", bufs=4, space="PSUM") as pt_pool,
            tc.tile_pool(name="acc", bufs=2, space="PSUM") as acc_pool,
            tc.tile_pool(name="out", bufs=2) as out_pool,
        ):
            ident = const_pool.tile([P, P], f32)
            make_identity(nc, ident[:])
            # fp32r matmul operands must be *produced* in fp32r (rounded) form,
            # so stage the f32 weights and convert once on-chip.
            wt_stage = const_pool.tile([P, KC, F], f32)
            nc.sync.dma_start(wt_stage[:], wt.rearrange("(c p) f -> p c f", p=P))
            wt_sb = const_pool.tile([P, KC, F], f32r)
            nc.vector.tensor_copy(wt_sb[:], wt_stage[:])

            a_r = a.rearrange("(s t p) (w f) -> s p t w f", t=T, p=P, f=W)
            for s in range(NSB):
                acc = acc_pool.tile([P, SBLK], f32)
                for w in range(NW):
                    slab = a_pool.tile([P, T, W], f32)
                    nc.sync.dma_start(slab[:], a_r[s, :, :, w, :])
                    for c in range(CW):
                        at = at_pool.tile([P, SBLK], f32r)
                        for t in range(T):
                            pt = pt_pool.tile([P, P], f32)
                            nc.tensor.transpose(
                                pt[:], slab[:, t, c * P : (c + 1) * P], ident[:]
                            )
                            if t % 2 == 0:
                                nc.vector.tensor_copy(at[:, t * P : (t + 1) * P], pt[:])
                            else:
                                nc.scalar.copy(at[:, t * P : (t + 1) * P], pt[:])
                        k = w * CW + c
                        nc.tensor.matmul(
                            acc[:],
                            wt_sb[:, k, :],
                            at[:],
                            start=(k == 0),
                            stop=(k == KC - 1),
                        )
                out_sb = out_pool.tile([P, SBLK], f32)
                nc.scalar.activation(
                    out_sb[:], acc[:], mybir.ActivationFunctionType.Ln
                )
                nc.scalar.mul(out_sb[:], out_sb[:], LOG10_SCALE)
                nc.sync.dma_start(o[:, s * SBLK : (s + 1) * SBLK], out_sb[:])

    nc.compile()
    return nc


def get_nc():
    if "nc" not in _CACHE:
        _CACHE["nc"] = _build_nc()
    return _CACHE["nc"]


def make_weighted_filter_t(trans_filter, lam):
    """(trans_filter * trapezoid_weights).T as contiguous [L, F] fp32."""
    lam = np.asarray(lam, np.float32)
    tf = np.asarray(trans_filter, np.float32)
    dx = np.diff(lam)
    w = np.zeros(L, np.float32)
    w[:-1] += 0.5 * dx
    w[1:] += 0.5 * dx
    return np.ascontiguousarray((tf * w[None, :]).T)


def make_in_maps(l_target, trans_filter, lam):
    a_full = np.ascontiguousarray(np.asarray(l_target, np.float32))
    wt = make_weighted_filter_t(trans_filter, lam)
    return [
        {"a": a_full[i * NB : (i + 1) * NB], "wt": wt} for i in range(N_CORES)
    ]


def kernel(l_target, trans_filter, lam, return_ph):
    rp = int(np.asarray(return_ph).reshape(()))
    if not rp:
        out = np.asarray(l_target, np.float32) * np.asarray(lam, np.float32)[None, :]
        return (out * np.float32(UNIT_F_NU)).astype(np.float32)

    from concourse.bass_utils import run_bass_kernel_spmd

    nc = get_nc()
    in_maps = make_in_maps(l_target, trans_filter, lam)
    res = run_bass_kernel_spmd(nc, in_maps, core_ids=list(range(N_CORES)))
    out = np.empty((B, F), np.float32)
    for i, r in enumerate(res.results):
        out[i * NB : (i + 1) * NB, :] = r["o"].T
    return out


# revision 5
# speedup vs baseline: 181.1807x; 181.1807x over previous
"""Trainium2 Bass kernel for the Converter photometry problem.

Computes out = -2.5*log10(l_target @ (trans_filter * w).T) where w are
trapezoid quadrature weights derived from lam.  Data-parallel over 8
NeuronCores: l_target is sharded along batch B; the (small) weighted
filter matrix is replicated.

Per-core device program:
  - WT = (trans_filter * w).T  [L, F] resident in SBUF (host-prepared).
  - A shard [1024, 8192] streamed in 4MB DMA slabs, natural [b, l] layout.
  - PE transposes each [128b x 128l] tile (exact fp32 pass-through) into
    PSUM; ACT/DVE copy it back to SBUF as [l, b].
  - fp32r matmuls (1 cycle/row at N=512) with the WT k-chunk stationary
    and transposed-A moving accumulate flux.T [128f, 512b] in PSUM.
  - Ln + scale on eviction; per-core output is out.T [F, 1024].
Host reassembles the full [B, F] output.
"""

import math

import numpy as np

B, L, F = 8192, 8192, 128
N_CORES = 8
NB = B // N_CORES  # batch rows per core
P = 128
SBLK = 512  # output super-block along b (PSUM free dim)
W = 2048  # l window per DMA slab
UNIT_F_NU = 1.0673e-02
LOG10_SCALE = -2.5 / math.log(10.0)

_CACHE = {}


def _build_nc(repeat=1):
    import concourse.bacc as bacc
    import concourse.mybir as mybir
    from concourse import tile
    from concourse.masks import make_identity

    f32 = mybir.dt.float32
    f32r = mybir.dt.float32r

    KC = L // P  # 64 contraction chunks
    NSB = NB // SBLK  # 2 output super-blocks
    NW = L // W  # 4 l-windows
    CW = W // P  # 16 chunks per window
    T = SBLK // P  # 4 row sub-blocks per super-block

    nc = bacc.Bacc(None, target_bir_lowering=False, debug=False)
    a = nc.dram_tensor("a", [NB, L], f32, kind="ExternalInput")
    wt = nc.dram_tensor("wt", [L, F], f32, kind="ExternalInput")
    o = nc.dram_tensor("o", [F, NB], f32, kind="ExternalOutput")

    with tile.TileContext(nc) as tc:
        with (
            tc.tile_pool(name="const", bufs=1) as const_pool,
            tc.tile_pool(name="a_slab", bufs=2) as a_pool,
            tc.tile_pool(name="at", bufs=3) as at_pool,
            tc.tile_pool(name="psum_t", bufs=4, space="PSUM") as pt_pool,
            tc.tile_pool(name="acc", bufs=2, space="PSUM") as acc_pool,
            tc.tile_pool(name="out", bufs=2) as out_pool,
        ):
            ident = const_pool.tile([P, P], f32)
            make_identity(nc, ident[:])
            # fp32r matmul operands must be *produced* in fp32r (rounded) form,
            # so stage the f32 weights and convert once on-chip.
            wt_stage = const_pool.tile([P, KC, F], f32)
            nc.sync.dma_start(wt_stage[:], wt.rearrange("(c p) f -> p c f", p=P))
            wt_sb = const_pool.tile([P, KC, F], f32r)
            nc.vector.tensor_copy(wt_sb[:], wt_stage[:])

            a_r = a.rearrange("(s t p) (w f) -> s p t w f", t=T, p=P, f=W)

            def body():
                for s in range(NSB):
                    acc = acc_pool.tile([P, SBLK], f32)
                    for w in range(NW):
                        slab = a_pool.tile([P, T, W], f32)
                        nc.sync.dma_start(slab[:], a_r[s, :, :, w, :])
                        for c in range(CW):
                            at = at_pool.tile([P, SBLK], f32r)
                            for t in range(T):
                                pt = pt_pool.tile([P, P], f32)
                                nc.tensor.transpose(
                                    pt[:], slab[:, t, c * P : (c + 1) * P], ident[:]
                                )
                                if t % 2 == 0:
                                    nc.vector.tensor_copy(
                                        at[:, t * P : (t + 1) * P], pt[:]
                                    )
                                else:
                                    nc.scalar.copy(at[:, t * P : (t + 1) * P], pt[:])
                            k = w * CW + c
                            nc.tensor.matmul(
                                acc[:],
                                wt_sb[:, k, :],
                                at[:],
                                start=(k == 0),
                                stop=(k == KC - 1),
                            )
                    out_sb = out_pool.tile([P, SBLK], f32)
                    nc.scalar.activation(
                        out_sb[:], acc[:], mybir.ActivationFunctionType.Ln
                    )
                    nc.scalar.mul(out_sb[:], out_sb[:], LOG10_SCALE)
                    nc.sync.dma_start(o[:, s * SBLK : (s + 1) * SBLK], out_sb[:])

            if repeat == 1:
                body()
            else:
                with tc.For_i(0, repeat, 1):
                    body()

    nc.compile()
    return nc


def get_nc():
    if "nc" not in _CACHE:
        _CACHE["nc"] = _build_nc()
    return _CACHE["nc"]


def make_weighted_filter_t(trans_filter, lam):
    """(trans_filter * trapezoid_weights).T as contiguous [L, F] fp32."""
    lam = np.asarray(lam, np.float32)
    tf = np.asarray(trans_filter, np.float32)
    dx = np.diff(lam)
    w = np.zeros(L, np.float32)
    w[:-1] += 0.5 * dx
    w[1:] += 0.5 * dx
    return np.ascontiguousarray((tf * w[None, :]).T)


def make_in_maps(l_target, trans_filter, lam):
    a_full = np.ascontiguousarray(np.asarray(l_target, np.float32))
    wt = make_weighted_filter_t(trans_filter, lam)
    return [
        {"a": a_full[i * NB : (i + 1) * NB], "wt": wt} for i in range(N_CORES)
    ]


def kernel(l_target, trans_filter, lam, return_ph):
    rp = int(np.asarray(return_ph).reshape(()))
    if not rp:
        out = np.asarray(l_target, np.float32) * np.asarray(lam, np.float32)[None, :]
        return (out * np.float32(UNIT_F_NU)).astype(np.float32)

    from concourse.bass_utils import run_bass_kernel_spmd

    nc = get_nc()
    in_maps = make_in_maps(l_target, trans_filter, lam)
    res = run_bass_kernel_spmd(nc, in_maps, core_ids=list(range(N_CORES)))
    out = np.empty((B, F), np.float32)
    for i, r in enumerate(res.results):
        out[i * NB : (i + 1) * NB, :] = r["o"].T
    return out


# revision 8
# speedup vs baseline: 243.0158x; 1.3413x over previous
"""Trainium2 Bass kernel for the Converter photometry problem.

Computes out = -2.5*log10(l_target @ (trans_filter * w).T) where w are
trapezoid quadrature weights derived from lam.  Data-parallel over 8
NeuronCores: l_target is sharded along batch B; the (small) weighted
filter matrix is replicated.

Per-core device program:
  - WT = (trans_filter * w).T  [L, F] resident in SBUF (host-prepared).
  - A shard [1024, 8192] streamed in 4MB DMA slabs, natural [b, l] layout.
  - PE transposes each [128b x 128l] tile (exact fp32 pass-through) into
    PSUM; ACT/DVE copy it back to SBUF as [l, b].
  - fp32r matmuls (1 cycle/row at N=512) with the WT k-chunk stationary
    and transposed-A moving accumulate flux.T [128f, 512b] in PSUM.
  - Ln + scale on eviction; per-core output is out.T [F, 1024].
Host reassembles the full [B, F] output.
"""

import math

import numpy as np

B, L, F = 8192, 8192, 128
N_CORES = 8
NB = B // N_CORES  # batch rows per core
P = 128
SBLK = 512  # output super-block along b (PSUM free dim)
W = 2048  # l window per DMA slab
UNIT_F_NU = 1.0673e-02
LOG10_SCALE = -2.5 / math.log(10.0)

_CACHE = {}


def _build_nc(repeat=1):
    import concourse.bacc as bacc
    import concourse.mybir as mybir
    from concourse import tile
    from concourse.masks import make_identity

    f32 = mybir.dt.float32
    f32r = mybir.dt.float32r

    KC = L // P  # 64 contraction chunks
    NSB = NB // SBLK  # 2 output super-blocks
    NW = L // W  # 4 l-windows
    CW = W // P  # 16 chunks per window
    T = SBLK // P  # 4 row sub-blocks per super-block

    nc = bacc.Bacc(None, target_bir_lowering=False, debug=False)
    # Declared f32r so DMAs are valid f32r producers (np side is float32;
    # f32r is bit-compatible, the PE rounds on read).
    a = nc.dram_tensor("a", [NB, L], f32r, kind="ExternalInput")
    wt = nc.dram_tensor("wt", [L, F], f32r, kind="ExternalInput")
    o = nc.dram_tensor("o", [F, NB], f32, kind="ExternalOutput")

    # Tapered l-windows: big DMA slabs in steady state, small final window so
    # the exposed tail (compute after the last DMA byte) is short.
    WINDOWS = [2048, 2048, 2048, 1024, 512, 512]
    assert sum(WINDOWS) == L

    with tile.TileContext(nc) as tc:
        with (
            tc.tile_pool(name="const", bufs=1) as const_pool,
            tc.tile_pool(name="a_slab", bufs=3) as a_pool,
            tc.tile_pool(name="at", bufs=4) as at_pool,
            tc.tile_pool(name="psum_t", bufs=3, space="PSUM") as pt_pool,
            tc.tile_pool(name="acc", bufs=2, space="PSUM") as acc_pool,
            tc.tile_pool(name="out", bufs=2) as out_pool,
        ):
            ident_f32 = const_pool.tile([P, P], f32)
            make_identity(nc, ident_f32[:])
            ident = const_pool.tile([P, P], f32r)
            nc.vector.tensor_copy(ident[:], ident_f32[:])
            # Weight load on the scalar HWDGE queue so it doesn't head-block
            # the first A slab on the sync queue.
            wt_sb = const_pool.tile([P, KC, F], f32r)
            nc.scalar.dma_start(wt_sb[:], wt.rearrange("(c p) f -> p c f", p=P))

            a_r = a.rearrange("(s t p) l -> s p t l", t=T, p=P)

            def body():
                for s in range(NSB):
                    acc = acc_pool.tile([P, SBLK], f32)
                    prev = None  # (at tile, k) software pipeline: matmul lags 1 chunk
                    off = 0
                    for wsz in WINDOWS:
                        slab = a_pool.tile([P, T, wsz], f32r, tag="slab")
                        nc.sync.dma_start(slab[:], a_r[s, :, :, off : off + wsz])
                        for c in range(wsz // P):
                            # One full PSUM bank collects all 4 transposes of
                            # this chunk; a single wide copy evicts it.
                            pt = pt_pool.tile([P, SBLK], f32r)
                            for t in range(T):
                                nc.tensor.transpose(
                                    pt[:, t * P : (t + 1) * P],
                                    slab[:, t, c * P : (c + 1) * P],
                                    ident[:],
                                )
                            at = at_pool.tile([P, SBLK], f32r)
                            if c % 2 == 0:
                                nc.vector.tensor_copy(at[:], pt[:])
                            else:
                                nc.scalar.copy(at[:], pt[:])
                            if prev is not None:
                                pat, pk = prev
                                nc.tensor.matmul(
                                    acc[:],
                                    wt_sb[:, pk, :],
                                    pat[:],
                                    start=(pk == 0),
                                    stop=False,
                                )
                            prev = (at, off // P + c)
                        off += wsz
                    pat, pk = prev
                    nc.tensor.matmul(
                        acc[:], wt_sb[:, pk, :], pat[:], start=False, stop=True
                    )
                    out_sb = out_pool.tile([P, SBLK], f32)
                    nc.scalar.activation(
                        out_sb[:], acc[:], mybir.ActivationFunctionType.Ln
                    )
                    nc.scalar.mul(out_sb[:], out_sb[:], LOG10_SCALE)
                    nc.scalar.dma_start(o[:, s * SBLK : (s + 1) * SBLK], out_sb[:])

            if repeat == 1:
                body()
            else:
                with tc.For_i(0, repeat, 1):
                    body()

    nc.compile()
    return nc


def get_nc():
    if "nc" not in _CACHE:
        _CACHE["nc"] = _build_nc()
    return _CACHE["nc"]


def make_weighted_filter_t(trans_filter, lam):
    """(trans_filter * trapezoid_weights).T as contiguous [L, F] fp32."""
    lam = np.asarray(lam, np.float32)
    tf = np.asarray(trans_filter, np.float32)
    dx = np.diff(lam)
    w = np.zeros(L, np.float32)
    w[:-1] += 0.5 * dx
    w[1:] += 0.5 * dx
    return np.ascontiguousarray((tf * w[None, :]).T)


def make_in_maps(l_target, trans_filter, lam):
    a_full = np.ascontiguousarray(np.asarray(l_target, np.float32))
    wt = make_weighted_filter_t(trans_filter, lam)
    return [
        {"a": a_full[i * NB : (i + 1) * NB], "wt": wt} for i in range(N_CORES)
    ]


def kernel(l_target, trans_filter, lam, return_ph):
    rp = int(np.asarray(return_ph).reshape(()))
    if not rp:
        out = np.asarray(l_target, np.float32) * np.asarray(lam, np.float32)[None, :]
        return (out * np.float32(UNIT_F_NU)).astype(np.float32)

    from concourse.bass_utils import run_bass_kernel_spmd

    nc = get_nc()
    in_maps = make_in_maps(l_target, trans_filter, lam)
    res = run_bass_kernel_spmd(nc, in_maps, core_ids=list(range(N_CORES)))
    out = np.empty((B, F), np.float32)
    for i, r in enumerate(res.results):
        out[i * NB : (i + 1) * NB, :] = r["o"].T
    return out


# revision 32
# speedup vs baseline: 349.8697x; 1.4397x over previous
"""Trainium2 Bass kernel for the Converter photometry problem.

Computes out = -2.5*log10(l_target @ (trans_filter * w).T) where w are
trapezoid quadrature weights derived from lam.  Data-parallel over 8
NeuronCores: l_target is sharded along batch B; the (small) weighted
filter matrix is replicated.

The GEMM contraction (L) must sit on SBUF partitions for the PE, but
l_target is [B, L] row-major, so every A tile needs a transpose.  fp32
DMA-transpose is unsupported, and this problem is memory-bound, so both
operands are carried as fp16 (output error ~2e-5 absolute vs the fp32
reference — the per-element rounding averages out over K=8192) which
halves HBM traffic AND makes the PE transpose 1 cycle/row:

  - WT = (trans_filter * w * 16).T [L, F] fp16, resident in SBUF.  The
    x16 keeps the smallest weights clear of the fp16 subnormal range;
    it is divided back out inside the Ln activation (input scale 1/16).
  - A shard [1024, 8192] fp16 streamed in ~1MB DMA slabs ([b, l] layout).
  - PE transposes each [128b x 128l] tile into one PSUM bank (4 chunks
    per bank); one wide DVE/ACT copy evicts it to SBUF as [l, b] fp16.
  - fp16 matmuls (1 cycle/row, N=512) with the WT k-chunk stationary and
    transposed-A moving accumulate flux.T [128f, 512b] in fp32 PSUM.
  - Ln(acc/16) + scale on eviction; per-core output is out.T [F, 1024].
Host reassembles the full [B, F] output.
"""

import math

import numpy as np

B, L, F = 8192, 8192, 128
N_CORES = 8
NB = B // N_CORES  # batch rows per core
P = 128
SBLK = 512  # output super-block along b (PSUM free dim)
UNIT_F_NU = 1.0673e-02
LOG10_SCALE = -2.5 / math.log(10.0)
WT_SCALE = 16.0

_CACHE = {}


def _build_nc(repeat=1):
    import concourse.bacc as bacc
    import concourse.mybir as mybir
    from concourse import tile
    from concourse.masks import make_identity

    f32 = mybir.dt.float32
    f16 = mybir.dt.float16

    KC = L // P  # 64 contraction chunks
    NSB = NB // SBLK  # 2 output super-blocks
    T = SBLK // P  # 4 row sub-blocks per super-block

    nc = bacc.Bacc(None, target_bir_lowering=False, debug=False)
    a = nc.dram_tensor("a", [NB, L], f16, kind="ExternalInput")
    # wt arrives host-rearranged to the on-chip [p, c, f] layout so the DMA
    # moves 16KB-contiguous runs per partition (full line rate).
    wt = nc.dram_tensor("wt", [P, KC * F], f16, kind="ExternalInput")
    o = nc.dram_tensor("o", [F, NB], f32, kind="ExternalOutput")

    # l-windows: short first windows for a fast head, uniform 1MB slabs in
    # between, short last window for a small tail.
    WINDOWS = [256, 512, 1024, 1024, 1024, 1024, 1024, 1024, 1024, 256]
    assert sum(WINDOWS) == L

    with tile.TileContext(nc) as tc:
        with (
            tc.tile_pool(name="const", bufs=1) as const_pool,
            tc.tile_pool(name="a_slab", bufs=8) as a_pool,
            tc.tile_pool(name="at", bufs=6) as at_pool,
            tc.tile_pool(name="psum_t", bufs=6, space="PSUM") as pt_pool,
            tc.tile_pool(name="acc", bufs=2, space="PSUM") as acc_pool,
            tc.tile_pool(name="out", bufs=2) as out_pool,
        ):
            ident_f32 = const_pool.tile([P, P], f32)
            make_identity(nc, ident_f32[:])
            ident = const_pool.tile([P, P], f16)
            nc.vector.tensor_copy(ident[:], ident_f32[:])
            wt_sb = const_pool.tile([P, KC, F], f16)
            warm = const_pool.tile([P, 1], f32)
            nc.gpsimd.memset(warm[:], 1.0)

            a_r = a.rearrange("(s t p) l -> s p t l", t=T, p=P)

            def body():
                for s in range(NSB):
                    acc = acc_pool.tile([P, SBLK], f32)
                    # Software pipeline: matmul lags 2 chunks so PE never
                    # waits on the PSUM->SBUF copy of its operand.
                    pending = []  # [(at tile, k), ...]
                    off = 0

                    def flush_one(last=False):
                        pat, pk = pending.pop(0)
                        nc.tensor.matmul(
                            acc[:],
                            wt_sb[:, pk, :],
                            pat[:],
                            start=(pk == 0),
                            stop=last,
                        )

                    for wi, wsz in enumerate(WINDOWS):
                        slab = a_pool.tile([P, T, wsz], f16, tag="slab")
                        nc.sync.dma_start(slab[:], a_r[s, :, :, off : off + wsz])
                        if s == 0 and wi == 1:
                            # Weight load: after the first two slab windows
                            # (so transposes start immediately) and on the
                            # scalar HWDGE queue (so slabs aren't blocked).
                            nc.scalar.dma_start(
                                wt_sb[:], wt.rearrange("p (c f) -> p c f", f=F)
                            )
                        for c in range(wsz // P):
                            # One PSUM bank collects all 4 transposes of this
                            # chunk; a single wide copy evicts it.
                            pt = pt_pool.tile([P, SBLK], f16)
                            for t in range(T):
                                nc.tensor.transpose(
                                    pt[:, t * P : (t + 1) * P],
                                    slab[:, t, c * P : (c + 1) * P],
                                    ident[:],
                                )
                            at = at_pool.tile([P, SBLK], f16)
                            # DVE gets 2x throughput on 16-bit copies, so all
                            # copies fit on DVE (46us < DMA's 60us) and ACT
                            # stays out of the per-chunk critical path.
                            nc.vector.tensor_copy(at[:], pt[:])
                            pending.append((at, off // P + c))
                            if len(pending) > 2:
                                flush_one()
                        off += wsz
                        if s == NSB - 1 and wi == len(WINDOWS) - 3:
                            # Warm ACT's Ln table early so the tail eviction
                            # never waits on LoadActFuncSet (~1.3us).
                            nc.scalar.activation(
                                warm[:], warm[:], mybir.ActivationFunctionType.Ln
                            )
                    while pending:
                        flush_one(last=(len(pending) == 1))
                    out_sb = out_pool.tile([P, SBLK], f32)
                    # Evict in halves so Ln/scale/DMA pipeline at the tail.
                    # Ln's input scale divides out the x16 weight pre-scale.
                    for h in range(2):
                        sl = slice(h * (SBLK // 2), (h + 1) * (SBLK // 2))
                        nc.scalar.activation(
                            out_sb[:, sl], acc[:, sl],
                            mybir.ActivationFunctionType.Ln,
                            scale=1.0 / WT_SCALE,
                        )
                        nc.vector.tensor_scalar_mul(
                            out_sb[:, sl], out_sb[:, sl], LOG10_SCALE
                        )
                        nc.scalar.dma_start(
                            o[:, s * SBLK + h * (SBLK // 2) :
                               s * SBLK + (h + 1) * (SBLK // 2)],
                            out_sb[:, sl],
                        )

            if repeat == 1:
                body()
            else:
                with tc.For_i(0, repeat, 1):
                    body()

    nc.compile()
    return nc


def get_nc():
    if "nc" not in _CACHE:
        _CACHE["nc"] = _build_nc()
    return _CACHE["nc"]


def make_weighted_filter_t(trans_filter, lam):
    """(trans_filter * trapz_weights * 16).T as fp16 in the on-chip
    [p, chunk, f] layout: element (p, c, f) = wt[c*128 + p, f]."""
    lam = np.asarray(lam, np.float32)
    tf = np.asarray(trans_filter, np.float32)
    dx = np.diff(lam)
    w = np.zeros(L, np.float32)
    w[:-1] += 0.5 * dx
    w[1:] += 0.5 * dx
    wt = (tf * (WT_SCALE * w)[None, :]).T.astype(np.float16)  # [L, F]
    return np.ascontiguousarray(
        wt.reshape(L // 128, 128, F).transpose(1, 0, 2).reshape(128, -1)
    )


def make_in_maps(l_target, trans_filter, lam):
    a_full = np.asarray(l_target, np.float32).astype(np.float16)
    wt = make_weighted_filter_t(trans_filter, lam)
    return [
        {"a": np.ascontiguousarray(a_full[i * NB : (i + 1) * NB]), "wt": wt}
        for i in range(N_CORES)
    ]


def kernel(l_target, trans_filter, lam, return_ph):
    rp = int(np.asarray(return_ph).reshape(()))
    if not rp:
        out = np.asarray(l_target, np.float32) * np.asarray(lam, np.float32)[None, :]
        return (out * np.float32(UNIT_F_NU)).astype(np.float32)

    from concourse.bass_utils import run_bass_kernel_spmd

    nc = get_nc()
    in_maps = make_in_maps(l_target, trans_filter, lam)
    res = run_bass_kernel_spmd(nc, in_maps, core_ids=list(range(N_CORES)))
    out = np.empty((B, F), np.float32)
    for i, r in enumerate(res.results):
        out[i * NB : (i + 1) * NB, :] = r["o"].T
    return out
